# revision 14
# baseline (speedup 1.0000x reference)
"""Trainium2 Bass kernel for nn_Attention2d.

Computation: GroupNorm(32 groups) -> 1x1 qkv conv -> 4-head attention over
H*W=4096 positions -> 1x1 proj conv -> residual add.

Sharding: one (batch, head) pair per NeuronCore (B=2 x NH=4 = 8 cores).
Each core:
  - GroupNorm of its batch slice (replicated across the 4 cores of a batch)
  - its head's q/k (with bias) and v^T (no bias)
  - S^T = k^T q in [keys-on-partitions, queries-on-free] layout
    (no max-subtraction: |S/8| <~ 6 so exp is safe in fp32)
  - P^T = exp(S^T/8); PV via matmul with lhsT = [v^T | ones]  -> the ones
    column yields the softmax denominators for free (row 64 of the output)
  - proj partial = Wp[:, head]^T @ PV_raw  (un-normalized)
Host: out[b] = x[b] + proj_bias + sum_h (partial_h / denom_h + Wp_h @ bv_h)
(per-query softmax normalization and the v-bias term commute through proj).

Matmul dtypes: the big attention matmuls (S, PV) and the qkv matmuls run as
float32r (1 cycle/row on the PE, vs 4 for plain fp32); the small proj /
GroupNorm matmuls stay exact fp32 so the softmax denominators and the final
projection add no extra rounding.
"""

import numpy as np

B, C, H, W = 2, 256, 64, 64
HW = H * W           # 4096
GROUPS = 32
NH = 4
HD = C // NH         # 64
EPS = 1e-5
P = 128
IB = 1024            # query block (PSUM-sized)
NIB = HW // IB       # 4
NJC = HW // P        # 32 key chunks
NCORES = B * NH

# "f32": exact fp32 everywhere (slow). "f32r": float32r qkv+attention
# (TF32-like rounding, ~3 cycles/row on HW). "f16": float16 operands for
# qkv/attention/proj matmuls (1 cycle/row + fast weight loads).
MM_MODE = "f16"

_module_cache = {}


def _build_module(mm=MM_MODE):
    import concourse.bacc as bacc
    import concourse.tile as tile
    import concourse.mybir as mybir

    dt = mybir.dt
    f32 = dt.float32
    AF = mybir.ActivationFunctionType
    OP = mybir.AluOpType
    # wdt: qkv-matmul operand dtype; adt: attention (S/PV) operand dtype
    if mm == "f32":
        wdt, adt = f32, f32
    elif mm == "f32r":
        wdt, adt = dt.float32r, dt.float32r
    elif mm == "f16":
        wdt, adt = dt.float16, dt.float16
    else:
        raise ValueError(mm)
    # proj matmul operand dtype: fp16 when fast (extra rounded copy of the
    # attention output), exact fp32 otherwise
    pdt = dt.float16 if mm == "f16" else f32

    nc = bacc.Bacc(trn_type="TRN2", target_bir_lowering=False, debug=False)

    # ---- DRAM I/O (per-core tensors; host prepares layouts) ----
    # channel layout everywhere: c = po*128 + pi  ->  [pi, po, ...]
    x_d = nc.dram_tensor("x", [P, 2, HW], f32, kind="ExternalInput").ap()
    wq_d = nc.dram_tensor("wq", [P, 2, HD], wdt, kind="ExternalInput").ap()
    wk_d = nc.dram_tensor("wk", [P, 2, HD], wdt, kind="ExternalInput").ap()
    wv_d = nc.dram_tensor("wv", [P, 2, HD], wdt, kind="ExternalInput").ap()
    wp_d = nc.dram_tensor("wp", [HD, C], pdt, kind="ExternalInput").ap()
    bq_d = nc.dram_tensor("bq", [HD, 1], f32, kind="ExternalInput").ap()
    bk_d = nc.dram_tensor("bk", [HD, 1], f32, kind="ExternalInput").ap()
    gnw_d = nc.dram_tensor("gnw", [P, 2, 1], f32, kind="ExternalInput").ap()
    gnb_d = nc.dram_tensor("gnb", [P, 2, 1], f32, kind="ExternalInput").ap()
    gmat_d = nc.dram_tensor("gmat", [P, 2, GROUPS], f32, kind="ExternalInput").ap()
    gbc_d = nc.dram_tensor("gbc", [GROUPS, 2, P], f32, kind="ExternalInput").ap()
    out_d = nc.dram_tensor("out", [P, 2, HW], f32, kind="ExternalOutput").ap()
    den_d = nc.dram_tensor("den", [NIB, IB], f32, kind="ExternalOutput").ap()

    with tile.TileContext(nc) as tc:
        with (
            tc.tile_pool(name="const", bufs=1) as const,
            tc.tile_pool(name="big", bufs=1) as big,
            tc.tile_pool(name="tmp", bufs=3) as tmp,
            tc.tile_pool(name="pt", bufs=3) as ptp,
            tc.tile_pool(name="oh", bufs=2) as ohp,
            tc.tile_pool(name="ostage", bufs=3) as ostage,
            tc.tile_pool(name="ps_st", bufs=2, space="PSUM") as ps_st,
            tc.tile_pool(name="ps_pv", bufs=1, space="PSUM") as ps_pv,
            tc.tile_pool(name="ps_sm", bufs=2, space="PSUM") as ps_sm,
        ):
            # ---- load everything ----
            x_sb = big.tile([P, 2, HW], f32)
            for po in range(2):
                for half in range(2):
                    hs = slice(half * (HW // 2), (half + 1) * (HW // 2))
                    nc.sync.dma_start(x_sb[:, po, hs], x_d[:, po, hs])
            wq_sb = const.tile([P, 2, HD], wdt)
            nc.sync.dma_start(wq_sb, wq_d)
            wk_sb = const.tile([P, 2, HD], wdt)
            nc.sync.dma_start(wk_sb, wk_d)
            wv_sb = const.tile([P, 2, HD], wdt)
            nc.sync.dma_start(wv_sb, wv_d)
            wp_sb = const.tile([HD, C], pdt)
            nc.sync.dma_start(wp_sb, wp_d)
            bq_sb = const.tile([HD, 1], f32)
            nc.sync.dma_start(bq_sb, bq_d)
            bk_sb = const.tile([HD, 1], f32)
            nc.sync.dma_start(bk_sb, bk_d)
            gnw_sb = const.tile([P, 2, 1], f32)
            nc.sync.dma_start(gnw_sb, gnw_d)
            gnb_sb = const.tile([P, 2, 1], f32)
            nc.sync.dma_start(gnb_sb, gnb_d)
            gmat_sb = const.tile([P, 2, GROUPS], f32)
            nc.sync.dma_start(gmat_sb, gmat_d)
            gbc_sb = const.tile([GROUPS, 2, P], f32)
            nc.sync.dma_start(gbc_sb, gbc_d)

            eps_sb = const.tile([GROUPS, 1], f32)
            nc.vector.memset(eps_sb, EPS)
            ones_sb = const.tile([P, 1], f32)
            nc.vector.memset(ones_sb, 1.0)

            # ---- GroupNorm statistics ----
            # per-channel mean/var via bn_stats, then [mean, E[x^2]] per channel
            mv = []
            for po in range(2):
                stats = tmp.tile([P, 8, 6], f32, tag="bnstats")
                for s in range(8):
                    nc.vector.bn_stats(
                        out=stats[:, s, :], in_=x_sb[:, po, s * 512:(s + 1) * 512]
                    )
                mvp = const.tile([P, 2], f32, tag=f"mv{po}")
                nc.vector.bn_aggr(out=mvp, in_=stats)
                msq = tmp.tile([P, 1], f32, tag="msq")
                nc.vector.tensor_mul(msq, mvp[:, 0:1], mvp[:, 0:1])
                nc.vector.tensor_add(mvp[:, 1:2], mvp[:, 1:2], msq)
                mv.append(mvp)

            # group-level [mean_g, E[x^2]_g] via indicator matmul (values 1/8)
            gst_ps = ps_sm.tile([GROUPS, 2], f32, tag="small")
            nc.tensor.matmul(gst_ps, lhsT=gmat_sb[:, 0, :], rhs=mv[0],
                             start=True, stop=False)
            nc.tensor.matmul(gst_ps, lhsT=gmat_sb[:, 1, :], rhs=mv[1],
                             start=False, stop=True)
            gst = const.tile([GROUPS, 2], f32)
            nc.vector.tensor_copy(gst, gst_ps)

            # var_g = E[x^2]_g - mean_g^2 ; rs = rsqrt(var+eps) via exp(-0.5*ln)
            # (keeps everything in the natural_log_exp ACT table set)
            varg = tmp.tile([GROUPS, 1], f32, tag="varg")
            nc.vector.tensor_mul(varg, gst[:, 0:1], gst[:, 0:1])
            nc.vector.tensor_sub(varg, gst[:, 1:2], varg)
            lnv = tmp.tile([GROUPS, 1], f32, tag="lnv")
            nc.scalar.activation(out=lnv, in_=varg, func=AF.Ln,
                                 bias=eps_sb, scale=1.0)
            st = const.tile([GROUPS, 2], f32)  # [rs_g, -mu_g*rs_g]
            nc.scalar.activation(out=st[:, 0:1], in_=lnv, func=AF.Exp, scale=-0.5)
            nc.vector.tensor_mul(st[:, 1:2], gst[:, 0:1], st[:, 0:1])
            nc.vector.tensor_scalar_mul(st[:, 1:2], st[:, 1:2], -1.0)

            # broadcast to channels, apply gn weight/bias; xn = s*x + t (-> wdt)
            xn_sb = big.tile([P, 2, HW], wdt)
            for po in range(2):
                stc_ps = ps_sm.tile([P, 2], f32, tag="small")
                nc.tensor.matmul(stc_ps, lhsT=gbc_sb[:, po, :], rhs=st,
                                 start=True, stop=True)
                sc = const.tile([P, 2], f32, tag=f"sca{po}")
                nc.vector.tensor_mul(sc[:, 0:1], stc_ps[:, 0:1], gnw_sb[:, po, :])
                nc.vector.tensor_mul(sc[:, 1:2], stc_ps[:, 1:2], gnw_sb[:, po, :])
                nc.vector.tensor_add(sc[:, 1:2], sc[:, 1:2], gnb_sb[:, po, :])
                for half in range(2):
                    hs = slice(half * (HW // 2), (half + 1) * (HW // 2))
                    nc.vector.tensor_scalar(
                        out=xn_sb[:, po, hs], in0=x_sb[:, po, hs],
                        scalar1=sc[:, 0:1], scalar2=sc[:, 1:2],
                        op0=OP.mult, op1=OP.add,
                    )

            # ---- qkv ----
            q_sb = big.tile([HD, HW], adt)
            k_sb = big.tile([HD, HW], adt)
            vt_sb = big.tile([P, NJC, HD + 1], adt)
            # ones column for the softmax-denominator row (produced by a
            # rounding DVE copy so the f32r/bf16 matmul verifier is happy)
            nc.vector.tensor_copy(
                vt_sb[:, :, HD:HD + 1],
                ones_sb[:, None, :].to_broadcast([P, NJC, 1]),
            )
            for n in range(HW // 512):
                ns = slice(n * 512, (n + 1) * 512)
                qp = ps_sm.tile([HD, 512], f32, tag="small")
                nc.tensor.matmul(qp, lhsT=wq_sb[:, 0, :],
                                 rhs=xn_sb[:, 0, ns], start=True, stop=False)
                nc.tensor.matmul(qp, lhsT=wq_sb[:, 1, :],
                                 rhs=xn_sb[:, 1, ns], start=False, stop=True)
                nc.vector.tensor_scalar_add(q_sb[:, ns], qp, bq_sb)
                kp = ps_sm.tile([HD, 512], f32, tag="small")
                nc.tensor.matmul(kp, lhsT=wk_sb[:, 0, :],
                                 rhs=xn_sb[:, 0, ns], start=True, stop=False)
                nc.tensor.matmul(kp, lhsT=wk_sb[:, 1, :],
                                 rhs=xn_sb[:, 1, ns], start=False, stop=True)
                nc.vector.tensor_scalar_add(k_sb[:, ns], kp, bk_sb)
            # v^T directly: [positions, head_dim], chunked by 128 positions
            for jc in range(NJC):
                js = slice(jc * P, (jc + 1) * P)
                vp = ps_sm.tile([P, HD], f32, tag="small")
                nc.tensor.matmul(vp, lhsT=xn_sb[:, 0, js],
                                 rhs=wv_sb[:, 0, :], start=True, stop=False)
                nc.tensor.matmul(vp, lhsT=xn_sb[:, 1, js],
                                 rhs=wv_sb[:, 1, :], start=False, stop=True)
                nc.vector.tensor_copy(vt_sb[:, jc, 0:HD], vp)

            # ---- attention + proj, blocked over queries ----
            SC = float(1.0 / np.sqrt(HD))
            for ib in range(NIB):
                ibs = ib * IB
                pv_ps = ps_pv.tile([HD + 1, IB], f32, tag="pv")
                for jc in range(NJC):
                    st_ps = ps_st.tile([P, IB], f32, tag="st")
                    for n2 in range(IB // 512):
                        nc.tensor.matmul(
                            st_ps[:, n2 * 512:(n2 + 1) * 512],
                            lhsT=k_sb[:, jc * P:(jc + 1) * P],
                            rhs=q_sb[:, ibs + n2 * 512: ibs + (n2 + 1) * 512],
                            start=True, stop=True,
                        )
                    pt = ptp.tile([P, IB], adt, tag="pt")
                    nc.scalar.activation(out=pt, in_=st_ps, func=AF.Exp, scale=SC)
                    for n2 in range(IB // 512):
                        nc.tensor.matmul(
                            pv_ps[:, n2 * 512:(n2 + 1) * 512],
                            lhsT=vt_sb[:, jc, :],
                            rhs=pt[:, n2 * 512:(n2 + 1) * 512],
                            start=(jc == 0), stop=(jc == NJC - 1),
                        )
                oh = ohp.tile([HD + 1, IB], f32, tag="oh")
                nc.vector.tensor_copy(oh, pv_ps)
                nc.sync.dma_start(den_d[ib:ib + 1, :], oh[HD:HD + 1, :])
                if pdt == f32:
                    ohp_mm = oh
                else:
                    ohp_mm = ohp.tile([HD, IB], pdt, tag="oh16")
                    nc.vector.tensor_copy(ohp_mm, oh[0:HD, :])
                for mt in range(2):
                    for n2 in range(IB // 512):
                        pp = ps_sm.tile([P, 512], f32, tag="small")
                        nc.tensor.matmul(
                            pp,
                            lhsT=wp_sb[:, mt * P:(mt + 1) * P],
                            rhs=ohp_mm[0:HD, n2 * 512:(n2 + 1) * 512],
                            start=True, stop=True,
                        )
                        sg = ostage.tile([P, 512], f32, tag="ostage")
                        nc.vector.tensor_copy(sg, pp)
                        nc.sync.dma_start(
                            out_d[:, mt, ibs + n2 * 512: ibs + (n2 + 1) * 512], sg
                        )
    nc.compile()
    return nc


def get_module(mm=MM_MODE):
    if mm not in _module_cache:
        _module_cache[mm] = _build_module(mm)
    return _module_cache[mm]


def _group_mats():
    gmat = np.zeros((P, 2, GROUPS), np.float32)
    gbc = np.zeros((GROUPS, 2, P), np.float32)
    for po in range(2):
        for pi in range(P):
            g = (po * P + pi) // 8
            gmat[pi, po, g] = 1.0 / 8.0
            gbc[g, po, pi] = 1.0
    return gmat, gbc


def make_in_maps(x, gn_weight, gn_bias, qkv_weight, qkv_bias,
                 proj_weight=None, mm=None):
    mm = mm or MM_MODE
    wp_np = np.float16 if mm == "f16" else np.float32
    x = np.asarray(x, np.float32)
    gn_weight = np.asarray(gn_weight, np.float32)
    gn_bias = np.asarray(gn_bias, np.float32)
    qkv_weight = np.asarray(qkv_weight, np.float32)
    qkv_bias = np.asarray(qkv_bias, np.float32)
    gmat, gbc = _group_mats()
    gnw = np.ascontiguousarray(gn_weight.reshape(2, P, 1).transpose(1, 0, 2))
    gnb = np.ascontiguousarray(gn_bias.reshape(2, P, 1).transpose(1, 0, 2))

    def wslice(row0):
        w = qkv_weight[row0:row0 + HD, :]            # [64, 256]
        return np.ascontiguousarray(
            w.T.reshape(2, P, HD).transpose(1, 0, 2).astype(wp_np))

    wps = [None] * NH
    if proj_weight is not None:
        pw = np.asarray(proj_weight, np.float32)
        wps = [np.ascontiguousarray(
            pw[:, h * HD:(h + 1) * HD].T.astype(wp_np)) for h in range(NH)]

    in_maps = []
    for b in range(B):
        xt = np.ascontiguousarray(x[b].reshape(2, P, HW).transpose(1, 0, 2))
        for h in range(NH):
            in_maps.append({
                "x": xt,
                "wq": wslice(h * HD),
                "wk": wslice(C + h * HD),
                "wv": wslice(2 * C + h * HD),
                "wp": wps[h],
                "bq": np.ascontiguousarray(
                    qkv_bias[h * HD:(h + 1) * HD].reshape(HD, 1)),
                "bk": np.ascontiguousarray(
                    qkv_bias[C + h * HD:C + (h + 1) * HD].reshape(HD, 1)),
                "gnw": gnw,
                "gnb": gnb,
                "gmat": gmat,
                "gbc": gbc,
            })
    return in_maps


def combine_outputs(results, x, qkv_bias, proj_weight, proj_bias):
    """results: list of 8 dicts with 'out' [128,2,HW] and 'den' [NIB,IB]."""
    x = np.asarray(x, np.float32)
    qkv_bias = np.asarray(qkv_bias, np.float32)
    proj_weight = np.asarray(proj_weight, np.float32)
    proj_bias = np.asarray(proj_bias, np.float32)
    y = np.empty((B, C, H, W), np.float32)
    for b in range(B):
        acc = x[b].reshape(C, HW) + proj_bias[:, None]
        for h in range(NH):
            r = results[b * NH + h]
            part = np.asarray(r["out"]).transpose(1, 0, 2).reshape(C, HW)
            den = np.asarray(r["den"]).reshape(HW)
            bv = qkv_bias[2 * C + h * HD: 2 * C + (h + 1) * HD]
            ch = proj_weight[:, h * HD:(h + 1) * HD] @ bv
            acc = acc + part / den[None, :] + ch[:, None]
        y[b] = acc.reshape(C, H, W)
    return y


def kernel(x, gn_weight, gn_bias, qkv_weight, qkv_bias, proj_weight, proj_bias):
    from concourse.bass_utils import run_bass_kernel_spmd

    nc = get_module()
    in_maps = make_in_maps(x, gn_weight, gn_bias, qkv_weight, qkv_bias,
                           proj_weight=proj_weight)
    res = run_bass_kernel_spmd(nc, in_maps, core_ids=list(range(NCORES)))
    return combine_outputs(res.results, x, qkv_bias, proj_weight, proj_bias)
